# revision 35
# baseline (speedup 1.0000x reference)
"""Trainium2 Bass kernel for nn_MultiHeadAttention (B=4,S=1024,D=1024,H=16,HD=64).

Sharding: 8 cores = 4 batches x 2 head-groups (tensor-parallel over heads).
Each core computes, for its (batch b, head-group g of 8 heads):
  - QKV projections with column-sharded Wq/Wk/Wv (A_local = 512)
  - attention scores / softmax / weights output for its 8 heads
  - context and a partial output projection with row-sharded Wo
Host gathers: weights concatenated over head-groups; out = sum of the two
partial projections per batch + bo.

Outputs match the reference: (out [B,S,D], weights [B,H,S,S]).
"""

import sys

try:
    import concourse  # noqa: F401
except ImportError:  # pragma: no cover
    sys.path.insert(0, "/opt/trn_rl_repo")

import numpy as np

import concourse.bass as bass
import concourse.mybir as mybir
import concourse.tile as tile
from concourse import bacc
from concourse.bass_utils import run_bass_kernel_spmd
from concourse.masks import make_identity

FP = mybir.dt.float32
HP = mybir.dt.float16
AF = mybir.ActivationFunctionType

B, S, D = 4, 1024, 1024
H, A = 16, 1024
HD = A // H          # 64
AL = A // 2          # 512 per head-group
HL = 8               # heads per core
N_CORES = 8
SCALE = 1.0 / np.sqrt(HD)

# test.py can flip these before calling kernel()
TRACE = False
TRACE_KW = {}
LAST_RESULT = None


def build(s=S):
    """Build the per-core Bass program. `s` (sequence length) is
    parameterized only so simulation tests can run a smaller version."""
    n_qt = s // 128          # q/k 128-row tiles
    n_sh = max(s // 512, 1)  # 512-wide column slabs
    sw = min(s, 512)         # slab width

    nc = bacc.Bacc("TRN2", target_bir_lowering=False, debug=False,
                   num_devices=N_CORES)

    xT = nc.dram_tensor("xT", [D, s], HP, kind="ExternalInput").ap()
    wq = nc.dram_tensor("wq", [D, AL], HP, kind="ExternalInput").ap()
    wk = nc.dram_tensor("wk", [D, AL], HP, kind="ExternalInput").ap()
    wv = nc.dram_tensor("wv", [D, AL], HP, kind="ExternalInput").ap()
    wo = nc.dram_tensor("wo", [AL, D], HP, kind="ExternalInput").ap()
    bq = nc.dram_tensor("bq", [AL], FP, kind="ExternalInput").ap()
    bk = nc.dram_tensor("bk", [AL], FP, kind="ExternalInput").ap()
    bv = nc.dram_tensor("bv", [AL], FP, kind="ExternalInput").ap()
    w_out = nc.dram_tensor("w_out", [HL, s, s], FP, kind="ExternalOutput").ap()
    o_out = nc.dram_tensor("o_out", [s, D], FP, kind="ExternalOutput").ap()
    ivf_d = nc.dram_tensor("ivf_d", [HL, s], FP).ap()  # internal scratch

    with tile.TileContext(nc) as tc:
        with tc.tile_pool(name="pers", bufs=1) as pers, \
             tc.tile_pool(name="psum", bufs=2, space="PSUM") as pp:

            # ---- persistent SBUF tensors ----
            QT = [pers.tile([128, s], HP, name=f"QT{i}", tag=f"QT{i}")
                  for i in range(4)]          # [a, s] chunks, a-chunk i
            KT = [pers.tile([128, s], HP, name=f"KT{i}", tag=f"KT{i}")
                  for i in range(4)]
            V = [pers.tile([128, AL], HP, name=f"V{i}", tag=f"V{i}")
                 for i in range(n_qt)]        # [s, a] chunks, s-chunk i
            WO = [pers.tile([128, D], HP, name=f"WO{i}", tag=f"WO{i}")
                  for i in range(4)]
            ctxT = [pers.tile([128, s], HP, name=f"cT{i}", tag=f"cT{i}")
                    for i in range(4)]        # [a, s] chunks (normalized)
            bq_sb = pers.tile([128, 4], FP, name="bq_sb", tag="bq_sb")
            bk_sb = pers.tile([128, 4], FP, name="bk_sb", tag="bk_sb")
            bv_bc = pers.tile([128, AL], FP, name="bv_bc", tag="bv_bc")
            ident = pers.tile([128, 128], FP, name="ident", tag="ident")
            make_identity(nc, ident)

            nc.sync.dma_start(out=bq_sb, in_=bq.rearrange("(a p) -> p a", p=128))
            nc.sync.dma_start(out=bk_sb, in_=bk.rearrange("(a p) -> p a", p=128))
            nc.sync.dma_start(out=bv_bc, in_=bv[None, :].to_broadcast([128, AL]))
            for i in range(4):
                nc.sync.dma_start(out=WO[i], in_=wo[i * 128:(i + 1) * 128, :])

            # ---- phase 1: projections ----
            with tc.tile_pool(name="ld", bufs=1) as ld:
                xs = [ld.tile([128, s], HP, name=f"xT{i}", tag=f"xT{i}")
                      for i in range(8)]
                qs = [ld.tile([128, AL], HP, name=f"wq{i}", tag=f"wq{i}")
                      for i in range(8)]
                ks = [ld.tile([128, AL], HP, name=f"wk{i}", tag=f"wk{i}")
                      for i in range(8)]
                vs = [ld.tile([128, AL], HP, name=f"wv{i}", tag=f"wv{i}")
                      for i in range(8)]
                for i in range(8):
                    nc.sync.dma_start(out=xs[i], in_=xT[i * 128:(i + 1) * 128, :])
                    nc.sync.dma_start(out=qs[i], in_=wq[i * 128:(i + 1) * 128, :])
                for i in range(8):
                    nc.sync.dma_start(out=ks[i], in_=wk[i * 128:(i + 1) * 128, :])
                    nc.sync.dma_start(out=vs[i], in_=wv[i * 128:(i + 1) * 128, :])

                # QT/KT: out[a,s] accumulating over d; lhsT = W chunk, rhs = xT
                for w_sb, b_sb, out_t in ((qs, bq_sb, QT), (ks, bk_sb, KT)):
                    for ca in range(4):
                        for sh in range(n_sh):
                            ps = pp.tile([128, sw], FP, tag="big", bufs=3,
                                         name="ps_pr")
                            for d in range(8):
                                nc.tensor.matmul(
                                    ps,
                                    lhsT=w_sb[d][:, ca * 128:(ca + 1) * 128],
                                    rhs=xs[d][:, sh * sw:(sh + 1) * sw],
                                    start=(d == 0), stop=(d == 7))
                            nc.vector.tensor_scalar_add(
                                out_t[ca][:, sh * sw:(sh + 1) * sw], ps,
                                b_sb[:, ca:ca + 1])
                # V: out[s,a] ; lhsT = xT chunk, rhs = Wv chunk
                for cs in range(n_qt):
                    ps = pp.tile([128, AL], FP, tag="big", bufs=3, name="ps_v")
                    for d in range(8):
                        nc.tensor.matmul(
                            ps, lhsT=xs[d][:, cs * 128:(cs + 1) * 128],
                            rhs=vs[d], start=(d == 0), stop=(d == 7))
                    nc.vector.tensor_add(V[cs], ps, bv_bc)

            # ---- phase 2: attention per head-pair ----
            with tc.tile_pool(name="st", bufs=1) as st:
                for pc in range(4):
                    sums_t = [st.tile([128, n_qt], FP, tag="sums", bufs=4,
                                      name="sums") for _ in range(2)]
                    inv_t = [st.tile([128, n_qt], FP, tag="inv", bufs=4,
                                     name="inv") for _ in range(2)]

                    # pass A: scores [q, kpos] -> exp/rowsum -> normalized W
                    for qt in range(n_qt):
                        for hh in range(2):
                            h = 2 * pc + hh
                            po = 64 * hh
                            ps_s = pp.tile([128, s], FP, tag="big", bufs=3,
                                           name="ps_s")
                            for kh in range(n_sh):
                                nc.tensor.matmul(
                                    ps_s[:, kh * sw:(kh + 1) * sw],
                                    lhsT=QT[pc][po:po + 64,
                                                qt * 128:(qt + 1) * 128],
                                    rhs=KT[pc][po:po + 64,
                                               kh * sw:(kh + 1) * sw],
                                    start=True, stop=True)
                            wu = st.tile([128, s], FP, tag="wu", bufs=4,
                                         name="wu")
                            nc.scalar.activation(
                                out=wu, in_=ps_s, func=AF.Exp, scale=SCALE,
                                accum_out=sums_t[hh][:, qt:qt + 1])
                            nc.vector.reciprocal(inv_t[hh][:, qt:qt + 1],
                                                 sums_t[hh][:, qt:qt + 1])
                            wn = st.tile([128, s], FP, tag="wn", bufs=4,
                                         name="wn")
                            # alternate normalize between DVE and the idle
                            # GpSimd so wu slot recycling never gates ScalarE
                            eng = nc.vector if (qt + hh) % 2 == 0 else nc.gpsimd
                            eng.tensor_scalar_mul(
                                wn, wu, inv_t[hh][:, qt:qt + 1])
                            nc.sync.dma_start(
                                out=w_out[h, qt * 128:(qt + 1) * 128, :],
                                in_=wn)

                    # inverse sums -> free-dim layout: PE transpose
                    # [128, n_qt] -> [n_qt, 128], one contiguous DRAM bounce,
                    # then partition-broadcast load to 64 rows
                    InvS = st.tile([128, s], FP, tag="invs", bufs=2,
                                   name="InvS")
                    for hh in range(2):
                        h = 2 * pc + hh
                        inv_ps = pp.tile([n_qt, 128], FP, tag="ctx", bufs=1,
                                         name="inv_ps")
                        nc.tensor.transpose(inv_ps, inv_t[hh], ident)
                        ivs = st.tile([n_qt, 128], FP, tag="ivs", bufs=4,
                                      name="ivs")
                        nc.vector.tensor_copy(ivs, inv_ps)
                        nc.sync.dma_start(out=ivf_d[h], in_=ivs)
                        nc.sync.dma_start(
                            out=InvS[64 * hh:64 * hh + 64, :],
                            in_=ivf_d[h][None, :].to_broadcast([64, s]))

                    # pass B per head (sequential so ctx accumulation groups
                    # never coexist in a PSUM bank): scores^T -> exp -> ctx^T
                    ps_ctx = pp.tile([128, s], FP, tag="ctx", bufs=1,
                                     name=f"ps_ctx{pc}")
                    for hh in range(2):
                        po = 64 * hh
                        a0 = (2 * pc + hh) * 64
                        for kt in range(n_qt):
                            ps_t = pp.tile([128, s], FP, tag="big", bufs=3,
                                           name="ps_t")
                            for qh in range(n_sh):
                                nc.tensor.matmul(
                                    ps_t[:, qh * sw:(qh + 1) * sw],
                                    lhsT=KT[pc][po:po + 64,
                                                kt * 128:(kt + 1) * 128],
                                    rhs=QT[pc][po:po + 64,
                                               qh * sw:(qh + 1) * sw],
                                    start=True, stop=True)
                            eT = st.tile([128, s], HP, tag="expT", bufs=6,
                                         name="eT")
                            nc.scalar.activation(out=eT, in_=ps_t,
                                                 func=AF.Exp, scale=SCALE)
                            for qh in range(n_sh):
                                nc.tensor.matmul(
                                    ps_ctx[po:po + 64,
                                           qh * sw:(qh + 1) * sw],
                                    lhsT=V[kt][:, a0:a0 + 64],
                                    rhs=eT[:, qh * sw:(qh + 1) * sw],
                                    start=(kt == 0), stop=(kt == n_qt - 1))

                    # normalize ctx^T by broadcast inverse sums
                    nc.vector.tensor_mul(ctxT[pc], ps_ctx, InvS)
                # ---- phase 3: partial out projection ----
                for stt in range(n_qt):
                    for dh in range(2):
                        ps_o = pp.tile([128, 512], FP, tag="big", bufs=3,
                                       name="ps_o")
                        for ca in range(4):
                            nc.tensor.matmul(
                                ps_o,
                                lhsT=ctxT[ca][:, stt * 128:(stt + 1) * 128],
                                rhs=WO[ca][:, dh * 512:(dh + 1) * 512],
                                start=(ca == 0), stop=(ca == 3))
                        ob = st.tile([128, 512], FP, tag="ob", bufs=4,
                                     name="ob")
                        nc.vector.tensor_copy(ob, ps_o)
                        nc.sync.dma_start(
                            out=o_out[stt * 128:(stt + 1) * 128,
                                      dh * 512:(dh + 1) * 512],
                            in_=ob)

    nc.compile()
    return nc


def make_in_maps(x, Wq, bq, Wk, bk, Wv, bv, Wo):
    x = np.asarray(x, dtype=np.float32)
    in_maps = []
    for c in range(N_CORES):
        b, g = c // 2, c % 2
        sl = slice(g * AL, (g + 1) * AL)
        in_maps.append({
            "xT": np.ascontiguousarray(x[b].T).astype(np.float16),
            "wq": np.ascontiguousarray(np.asarray(Wq)[:, sl]).astype(np.float16),
            "wk": np.ascontiguousarray(np.asarray(Wk)[:, sl]).astype(np.float16),
            "wv": np.ascontiguousarray(np.asarray(Wv)[:, sl]).astype(np.float16),
            "wo": np.ascontiguousarray(np.asarray(Wo)[sl, :]).astype(np.float16),
            "bq": np.ascontiguousarray(np.asarray(bq)[sl]),
            "bk": np.ascontiguousarray(np.asarray(bk)[sl]),
            "bv": np.ascontiguousarray(np.asarray(bv)[sl]),
        })
    return in_maps


_NC = None


def kernel(x, Wq, bq, Wk, bk, Wv, bv, Wo, bo):
    global _NC, LAST_RESULT
    if _NC is None:
        _NC = build()
    in_maps = make_in_maps(x, Wq, bq, Wk, bk, Wv, bv, Wo)
    r = run_bass_kernel_spmd(_NC, in_maps, list(range(N_CORES)),
                             trace=TRACE, **TRACE_KW)
    LAST_RESULT = r

    weights = np.empty((B, H, S, S), dtype=np.float32)
    out = np.empty((B, S, D), dtype=np.float32)
    bo = np.asarray(bo, dtype=np.float32)
    for c in range(N_CORES):
        b, g = c // 2, c % 2
        weights[b, g * HL:(g + 1) * HL] = r.results[c]["w_out"].astype(np.float32)
    for b in range(B):
        out[b] = (r.results[2 * b]["o_out"].astype(np.float32)
                  + r.results[2 * b + 1]["o_out"].astype(np.float32) + bo)
    return out, weights


# revision 36
# speedup vs baseline: 2.5726x; 2.5726x over previous
"""Trainium2 Bass kernel for nn_MultiHeadAttention (B=4,S=1024,D=1024,H=16,HD=64).

Sharding: 8 cores = 4 batches x 2 head-groups (tensor-parallel over heads).
Each core computes, for its (batch b, head-group g of 8 heads):
  - QKV projections with column-sharded Wq/Wk/Wv (A_local = 512)
  - attention scores / softmax / weights output for its 8 heads
  - context and a partial output projection with row-sharded Wo
Host gathers: weights concatenated over head-groups; out = sum of the two
partial projections per batch + bo.

Outputs match the reference: (out [B,S,D], weights [B,H,S,S]).
"""

import sys

try:
    import concourse  # noqa: F401
except ImportError:  # pragma: no cover
    sys.path.insert(0, "/opt/trn_rl_repo")

import numpy as np

import concourse.bass as bass
import concourse.mybir as mybir
import concourse.tile as tile
from concourse import bacc
from concourse.bass_utils import run_bass_kernel_spmd
from concourse.masks import make_identity

FP = mybir.dt.float32
HP = mybir.dt.float16
AF = mybir.ActivationFunctionType

B, S, D = 4, 1024, 1024
H, A = 16, 1024
HD = A // H          # 64
AL = A // 2          # 512 per head-group
HL = 8               # heads per core
N_CORES = 8
SCALE = 1.0 / np.sqrt(HD)

# test.py can flip these before calling kernel()
TRACE = False
TRACE_KW = {}
LAST_RESULT = None


def build(s=S):
    """Build the per-core Bass program. `s` (sequence length) is
    parameterized only so simulation tests can run a smaller version."""
    n_qt = s // 128          # q/k 128-row tiles
    n_sh = max(s // 512, 1)  # 512-wide column slabs
    sw = min(s, 512)         # slab width

    nc = bacc.Bacc("TRN2", target_bir_lowering=False, debug=False,
                   num_devices=N_CORES)

    xT = nc.dram_tensor("xT", [D, s], HP, kind="ExternalInput").ap()
    wq = nc.dram_tensor("wq", [D, AL], HP, kind="ExternalInput").ap()
    wk = nc.dram_tensor("wk", [D, AL], HP, kind="ExternalInput").ap()
    wv = nc.dram_tensor("wv", [D, AL], HP, kind="ExternalInput").ap()
    wo = nc.dram_tensor("wo", [AL, D], HP, kind="ExternalInput").ap()
    bq = nc.dram_tensor("bq", [AL], FP, kind="ExternalInput").ap()
    bk = nc.dram_tensor("bk", [AL], FP, kind="ExternalInput").ap()
    bv = nc.dram_tensor("bv", [AL], FP, kind="ExternalInput").ap()
    w_out = nc.dram_tensor("w_out", [HL, s, s], FP, kind="ExternalOutput").ap()
    o_out = nc.dram_tensor("o_out", [s, D], FP, kind="ExternalOutput").ap()
    ivf_d = nc.dram_tensor("ivf_d", [HL, s], FP).ap()  # internal scratch

    with tile.TileContext(nc) as tc:
        with tc.tile_pool(name="pers", bufs=1) as pers, \
             tc.tile_pool(name="psum", bufs=2, space="PSUM") as pp:

            # ---- persistent SBUF tensors ----
            QT = [pers.tile([128, s], HP, name=f"QT{i}", tag=f"QT{i}")
                  for i in range(4)]          # [a, s] chunks, a-chunk i
            KT = [pers.tile([128, s], HP, name=f"KT{i}", tag=f"KT{i}")
                  for i in range(4)]
            V = [pers.tile([128, AL], HP, name=f"V{i}", tag=f"V{i}")
                 for i in range(n_qt)]        # [s, a] chunks, s-chunk i
            WO = [pers.tile([128, D], HP, name=f"WO{i}", tag=f"WO{i}")
                  for i in range(4)]
            ctxT = [pers.tile([128, s], HP, name=f"cT{i}", tag=f"cT{i}")
                    for i in range(4)]        # [a, s] chunks (normalized)
            bq_sb = pers.tile([128, 4], FP, name="bq_sb", tag="bq_sb")
            bk_sb = pers.tile([128, 4], FP, name="bk_sb", tag="bk_sb")
            bv_bc = pers.tile([128, AL], FP, name="bv_bc", tag="bv_bc")
            ident = pers.tile([128, 128], FP, name="ident", tag="ident")
            make_identity(nc, ident)

            nc.sync.dma_start(out=bq_sb, in_=bq.rearrange("(a p) -> p a", p=128))
            nc.sync.dma_start(out=bk_sb, in_=bk.rearrange("(a p) -> p a", p=128))
            nc.sync.dma_start(out=bv_bc, in_=bv[None, :].to_broadcast([128, AL]))
            for i in range(4):
                nc.sync.dma_start(out=WO[i], in_=wo[i * 128:(i + 1) * 128, :])

            # ---- phase 1: projections ----
            with tc.tile_pool(name="ld", bufs=1) as ld:
                xs = [ld.tile([128, s], HP, name=f"xT{i}", tag=f"xT{i}")
                      for i in range(8)]
                qs = [ld.tile([128, AL], HP, name=f"wq{i}", tag=f"wq{i}")
                      for i in range(8)]
                ks = [ld.tile([128, AL], HP, name=f"wk{i}", tag=f"wk{i}")
                      for i in range(8)]
                vs = [ld.tile([128, AL], HP, name=f"wv{i}", tag=f"wv{i}")
                      for i in range(8)]
                for i in range(8):
                    nc.sync.dma_start(out=xs[i], in_=xT[i * 128:(i + 1) * 128, :])
                    nc.sync.dma_start(out=qs[i], in_=wq[i * 128:(i + 1) * 128, :])
                for i in range(8):
                    nc.sync.dma_start(out=ks[i], in_=wk[i * 128:(i + 1) * 128, :])
                    nc.sync.dma_start(out=vs[i], in_=wv[i * 128:(i + 1) * 128, :])

                # QT/KT: out[a,s] accumulating over d; lhsT = W chunk, rhs = xT
                for w_sb, b_sb, out_t in ((qs, bq_sb, QT), (ks, bk_sb, KT)):
                    for ca in range(4):
                        for sh in range(n_sh):
                            ps = pp.tile([128, sw], FP, tag="big", bufs=3,
                                         name="ps_pr")
                            for d in range(8):
                                nc.tensor.matmul(
                                    ps,
                                    lhsT=w_sb[d][:, ca * 128:(ca + 1) * 128],
                                    rhs=xs[d][:, sh * sw:(sh + 1) * sw],
                                    start=(d == 0), stop=(d == 7))
                            nc.vector.tensor_scalar_add(
                                out_t[ca][:, sh * sw:(sh + 1) * sw], ps,
                                b_sb[:, ca:ca + 1])
                # V: out[s,a] ; lhsT = xT chunk, rhs = Wv chunk
                for cs in range(n_qt):
                    ps = pp.tile([128, AL], FP, tag="big", bufs=3, name="ps_v")
                    for d in range(8):
                        nc.tensor.matmul(
                            ps, lhsT=xs[d][:, cs * 128:(cs + 1) * 128],
                            rhs=vs[d], start=(d == 0), stop=(d == 7))
                    nc.vector.tensor_add(V[cs], ps, bv_bc)

            # ---- phase 2: attention per head-pair ----
            with tc.tile_pool(name="st", bufs=1) as st:
                for pc in range(4):
                    sums_t = [st.tile([128, n_qt], FP, tag="sums", bufs=4,
                                      name="sums") for _ in range(2)]
                    inv_t = [st.tile([128, n_qt], FP, tag="inv", bufs=4,
                                     name="inv") for _ in range(2)]

                    # pass A: scores [q, kpos] -> exp/rowsum -> normalized W
                    for qt in range(n_qt):
                        for hh in range(2):
                            h = 2 * pc + hh
                            po = 64 * hh
                            ps_s = pp.tile([128, s], FP, tag="big", bufs=3,
                                           name="ps_s")
                            for kh in range(n_sh):
                                nc.tensor.matmul(
                                    ps_s[:, kh * sw:(kh + 1) * sw],
                                    lhsT=QT[pc][po:po + 64,
                                                qt * 128:(qt + 1) * 128],
                                    rhs=KT[pc][po:po + 64,
                                               kh * sw:(kh + 1) * sw],
                                    start=True, stop=True)
                            wu = st.tile([128, s], FP, tag="wu", bufs=4,
                                         name="wu")
                            nc.scalar.activation(
                                out=wu, in_=ps_s, func=AF.Exp, scale=SCALE,
                                accum_out=sums_t[hh][:, qt:qt + 1])
                            nc.vector.reciprocal(inv_t[hh][:, qt:qt + 1],
                                                 sums_t[hh][:, qt:qt + 1])
                            wn = st.tile([128, s], FP, tag="wn", bufs=4,
                                         name="wn")
                            nc.vector.tensor_scalar_mul(
                                wn, wu, inv_t[hh][:, qt:qt + 1])
                            nc.sync.dma_start(
                                out=w_out[h, qt * 128:(qt + 1) * 128, :],
                                in_=wn)

                    # inverse sums -> free-dim layout: PE transpose
                    # [128, n_qt] -> [n_qt, 128], one contiguous DRAM bounce,
                    # then partition-broadcast load to 64 rows
                    InvS = st.tile([128, s], FP, tag="invs", bufs=2,
                                   name="InvS")
                    for hh in range(2):
                        h = 2 * pc + hh
                        inv_ps = pp.tile([n_qt, 128], FP, tag="ctx", bufs=1,
                                         name="inv_ps")
                        nc.tensor.transpose(inv_ps, inv_t[hh], ident)
                        ivs = st.tile([n_qt, 128], FP, tag="ivs", bufs=4,
                                      name="ivs")
                        nc.vector.tensor_copy(ivs, inv_ps)
                        nc.sync.dma_start(out=ivf_d[h], in_=ivs)
                        nc.sync.dma_start(
                            out=InvS[64 * hh:64 * hh + 64, :],
                            in_=ivf_d[h][None, :].to_broadcast([64, s]))

                    # pass B per head (sequential so ctx accumulation groups
                    # never coexist in a PSUM bank): scores^T -> exp -> ctx^T
                    ps_ctx = pp.tile([128, s], FP, tag="ctx", bufs=1,
                                     name=f"ps_ctx{pc}")
                    for hh in range(2):
                        po = 64 * hh
                        a0 = (2 * pc + hh) * 64
                        for kt in range(n_qt):
                            ps_t = pp.tile([128, s], FP, tag="big", bufs=3,
                                           name="ps_t")
                            for qh in range(n_sh):
                                nc.tensor.matmul(
                                    ps_t[:, qh * sw:(qh + 1) * sw],
                                    lhsT=KT[pc][po:po + 64,
                                                kt * 128:(kt + 1) * 128],
                                    rhs=QT[pc][po:po + 64,
                                               qh * sw:(qh + 1) * sw],
                                    start=True, stop=True)
                            eT = st.tile([128, s], HP, tag="expT", bufs=6,
                                         name="eT")
                            nc.scalar.activation(out=eT, in_=ps_t,
                                                 func=AF.Exp, scale=SCALE)
                            for qh in range(n_sh):
                                nc.tensor.matmul(
                                    ps_ctx[po:po + 64,
                                           qh * sw:(qh + 1) * sw],
                                    lhsT=V[kt][:, a0:a0 + 64],
                                    rhs=eT[:, qh * sw:(qh + 1) * sw],
                                    start=(kt == 0), stop=(kt == n_qt - 1))

                    # normalize ctx^T by broadcast inverse sums
                    nc.vector.tensor_mul(ctxT[pc], ps_ctx, InvS)
                # ---- phase 3: partial out projection ----
                for stt in range(n_qt):
                    for dh in range(2):
                        ps_o = pp.tile([128, 512], FP, tag="big", bufs=3,
                                       name="ps_o")
                        for ca in range(4):
                            nc.tensor.matmul(
                                ps_o,
                                lhsT=ctxT[ca][:, stt * 128:(stt + 1) * 128],
                                rhs=WO[ca][:, dh * 512:(dh + 1) * 512],
                                start=(ca == 0), stop=(ca == 3))
                        ob = st.tile([128, 512], FP, tag="ob", bufs=4,
                                     name="ob")
                        nc.vector.tensor_copy(ob, ps_o)
                        nc.sync.dma_start(
                            out=o_out[stt * 128:(stt + 1) * 128,
                                      dh * 512:(dh + 1) * 512],
                            in_=ob)

    nc.compile()
    return nc


def make_in_maps(x, Wq, bq, Wk, bk, Wv, bv, Wo):
    x = np.asarray(x, dtype=np.float32)
    in_maps = []
    for c in range(N_CORES):
        b, g = c // 2, c % 2
        sl = slice(g * AL, (g + 1) * AL)
        in_maps.append({
            "xT": np.ascontiguousarray(x[b].T).astype(np.float16),
            "wq": np.ascontiguousarray(np.asarray(Wq)[:, sl]).astype(np.float16),
            "wk": np.ascontiguousarray(np.asarray(Wk)[:, sl]).astype(np.float16),
            "wv": np.ascontiguousarray(np.asarray(Wv)[:, sl]).astype(np.float16),
            "wo": np.ascontiguousarray(np.asarray(Wo)[sl, :]).astype(np.float16),
            "bq": np.ascontiguousarray(np.asarray(bq)[sl]),
            "bk": np.ascontiguousarray(np.asarray(bk)[sl]),
            "bv": np.ascontiguousarray(np.asarray(bv)[sl]),
        })
    return in_maps


_NC = None


def kernel(x, Wq, bq, Wk, bk, Wv, bv, Wo, bo):
    global _NC, LAST_RESULT
    if _NC is None:
        _NC = build()
    in_maps = make_in_maps(x, Wq, bq, Wk, bk, Wv, bv, Wo)
    r = run_bass_kernel_spmd(_NC, in_maps, list(range(N_CORES)),
                             trace=TRACE, **TRACE_KW)
    LAST_RESULT = r

    weights = np.empty((B, H, S, S), dtype=np.float32)
    out = np.empty((B, S, D), dtype=np.float32)
    bo = np.asarray(bo, dtype=np.float32)
    for c in range(N_CORES):
        b, g = c // 2, c % 2
        weights[b, g * HL:(g + 1) * HL] = r.results[c]["w_out"].astype(np.float32)
    for b in range(B):
        out[b] = (r.results[2 * b]["o_out"].astype(np.float32)
                  + r.results[2 * b + 1]["o_out"].astype(np.float32) + bo)
    return out, weights
